# revision 24
# baseline (speedup 1.0000x reference)
"""Trainium2 Bass kernel for EncoderVQVAEFixed (VQ-VAE forward pass).

Shards the 27000-dim encoder contraction / decoder output dim across 8
NeuronCores (tensor-parallel), all-reduces the tiny latent z, replicates the
VQ + first decoder layer, and re-shards the big decoder matmul column-wise.

The two large matmuls (x @ W_enc and h @ W_d2) run as 3-pass float32r
split-precision products (a=ar+ac, b=br+bc; out = ar@br + ar@bc + ac@br),
which matches fp32 accuracy (~2e-7) at 3 PE cycles/row instead of fp32's 4.
"""

import os
import sys

sys.path.insert(0, "/opt/trn_rl_repo")

import numpy as np

import concourse.bass as bass
import concourse.bacc as bacc
import concourse.tile as tile
from concourse import mybir
from concourse.bass_utils import run_bass_kernel_spmd
from concourse.masks import make_identity

NCORES = 8
B = 256
LEADS, T = 12, 2250
IN_DIM = LEADS * T          # 27000
ENC = 768
LAT = 128
K = 512
COMMIT = 0.1
SLICE = IN_DIM // NCORES    # 3375
KC = 27                     # contraction tiles per core
KP = 125                    # partition size of each contraction tile (27*125=3375)
WENC_CHUNK = 3              # kc per wenc DMA chunk (9 chunks)
# last chunk is 303 wide; f32r matmuls need aligned free dims, so pad to 304
# with a zeroed column (nn = padded width, nv = valid width)
N_CHUNKS = [(0, 512, 512), (512, 512, 512), (1024, 512, 512), (1536, 512, 512),
            (2048, 512, 512), (2560, 512, 512), (3072, 304, 303)]
F32 = mybir.dt.float32
F32R = mybir.dt.float32r
ENC_SPLIT = os.environ.get("ENC_SPLIT", "1") == "1"
DEC_SPLIT = os.environ.get("DEC_SPLIT", "1") == "1"


def build_nc(with_collective=True):
    nc = bacc.Bacc(None, target_bir_lowering=False, debug=False, num_devices=NCORES)

    x_in = nc.dram_tensor("x_slice", [B, SLICE], F32, kind="ExternalInput")
    xt_in = nc.dram_tensor("xt_slice", [SLICE, B], F32, kind="ExternalInput")
    wenc_in = nc.dram_tensor("wenc_slice", [SLICE, ENC], F32, kind="ExternalInput")
    wlat_in = nc.dram_tensor("wlat", [ENC, LAT], F32, kind="ExternalInput")
    blat_in = nc.dram_tensor("blat", [1, LAT], F32, kind="ExternalInput")
    cb_in = nc.dram_tensor("codebook", [K, LAT], F32, kind="ExternalInput")
    wd1_in = nc.dram_tensor("wd1", [LAT, ENC], F32, kind="ExternalInput")
    bd1_in = nc.dram_tensor("bd1", [1, ENC], F32, kind="ExternalInput")
    wd2_in = nc.dram_tensor("wd2_slice", [ENC, SLICE], F32, kind="ExternalInput")
    bd2_in = nc.dram_tensor("bd2_slice", [1, SLICE], F32, kind="ExternalInput")

    xrt_out = nc.dram_tensor("xrt", [SLICE, B], F32, kind="ExternalOutput")
    idx_out = nc.dram_tensor("indices", [B], mybir.dt.int32, kind="ExternalOutput")
    rsse_out = nc.dram_tensor("recon_sse", [1, 1], F32, kind="ExternalOutput")
    vsse_out = nc.dram_tensor("vq_sse", [1, 1], F32, kind="ExternalOutput")

    with tile.TileContext(nc) as tc:
        with (
            tc.tile_pool(name="const", bufs=1) as cpool,
            tc.tile_pool(name="xp", bufs=1) as xpool,
            tc.tile_pool(name="work", bufs=1) as work,
            tc.tile_pool(name="rot", bufs=2) as rot,
            tc.tile_pool(name="wd2f", bufs=4) as wd2f,
            tc.tile_pool(name="vqp", bufs=1) as vqp,
            tc.tile_pool(name="ps_small", bufs=4, space="PSUM") as ps_small,
            tc.tile_pool(name="dram", bufs=1, space="DRAM") as dram,
        ):
            # ---------------- constants / small inputs ----------------
            ident = cpool.tile([128, 128], F32, tag="ident")
            make_identity(nc, ident[:])
            ones_col = cpool.tile([128, 1], F32, tag="ones_col")
            nc.gpsimd.memset(ones_col[:], 1.0)
            ones_row = cpool.tile([1, 128], F32, tag="ones_row")
            nc.gpsimd.memset(ones_row[:], 1.0)
            one1 = cpool.tile([1, 1], F32, tag="one1")
            nc.gpsimd.memset(one1[:], 1.0)

            wlat_sb = cpool.tile([128, 6, LAT], F32, tag="wlat")
            nc.sync.dma_start(
                wlat_sb[:], wlat_in[:].rearrange("(c p) l -> p c l", p=128)
            )
            blat_sb = cpool.tile([1, LAT], F32, tag="blat")
            nc.sync.dma_start(blat_sb[:], blat_in[:])
            cb_sb = cpool.tile([128, 4, LAT], F32, tag="cb")
            nc.sync.dma_start(cb_sb[:], cb_in[:].rearrange("(c p) l -> p c l", p=128))
            wd1_sb = cpool.tile([LAT, ENC], F32, tag="wd1")
            nc.sync.dma_start(wd1_sb[:], wd1_in[:])
            bd1_sb = cpool.tile([1, ENC], F32, tag="bd1")
            nc.sync.dma_start(bd1_sb[:], bd1_in[:])
            # b_d2 as per-partition columns for the transposed decoder output
            bd2c_sb = cpool.tile([KP, KC], F32, tag="bd2c")
            nc.gpsimd.dma_start(
                bd2c_sb[:], bd2_in[:].rearrange("o (c p) -> (o p) c", p=KP))


            # b_d1 as per-partition columns (independent of everything else)
            bd1c = work.tile([128, 6], F32, tag="bd1c")
            for m in range(6):
                ps_b = ps_small.tile([128, 1], F32, tag="small", name=f"ps_bd1_{m}")
                nc.tensor.matmul(ps_b[:], bd1_sb[0:1, m * 128:(m + 1) * 128],
                                 one1[0:1, 0:1], start=True, stop=True)
                nc.vector.tensor_copy(bd1c[:, m:m + 1], ps_b[:])

            # loss accumulators
            racc = work.tile([128, KC], F32, tag="racc")
            vacc = work.tile([128, 2], F32, tag="vacc")

            # codebook transpose + e2 row: independent of z, runs during encoder
            cbT_sb = vqp.tile([128, K], F32, tag="cbT")
            for c in range(4):
                ps_t = ps_small.tile([128, 128], F32, tag="small",
                                     name=f"ps_cbt{c}")
                nc.tensor.transpose(ps_t[:], cb_sb[:, c, :], ident[:])
                nc.scalar.copy(cbT_sb[:, c * 128:(c + 1) * 128], ps_t[:])
            cbsqT_sb = vqp.tile([128, K], F32, tag="cbsqT")
            nc.vector.tensor_tensor(out=cbsqT_sb[:], in0=cbT_sb[:],
                                    in1=cbT_sb[:], op=mybir.AluOpType.mult)
            ps_e2 = ps_small.tile([1, K], F32, tag="small", name="ps_e2")
            nc.tensor.matmul(ps_e2[:], ones_col[:, 0:1], cbsqT_sb[:],
                             start=True, stop=True)
            nege2_sb = vqp.tile([1, K], F32, tag="nege2")
            nc.scalar.mul(nege2_sb[:], ps_e2[:], -1.0)

            # ---------------- encoder: featT[768, 256], 3-pass f32r ----------
            zTp_sb = work.tile([128, B], F32, tag="zTp")
            with (
                tc.tile_pool(name="wencf", bufs=2) as wencf,
                tc.tile_pool(name="wencr", bufs=3) as wencr,
                tc.tile_pool(name="wencc", bufs=3) as wencc,
                tc.tile_pool(name="xtf", bufs=2) as xtfp,
                tc.tile_pool(name="xtr", bufs=2) as xtrp,
                tc.tile_pool(name="xtc", bufs=2) as xtcp,
                tc.tile_pool(name="featp", bufs=1) as featp,
                tc.tile_pool(name="ps_feat", bufs=1, space="PSUM") as ps_feat,
            ):
                # 6 logical [128, 256] accumulators packed 2-per-bank
                feat_banks = [
                    ps_feat.tile([128, 512], F32, tag=f"fb{i}", name=f"feat_bank{i}")
                    for i in range(3)
                ]
                feat_ps = [
                    feat_banks[m // 2][:, (m % 2) * 256:(m % 2) * 256 + 256]
                    for m in range(6)
                ]
                # small leading groups so the first matmuls start early
                groups = [1, 2] + [3] * 8
                base = 0
                for g, nkc in enumerate(groups):
                    wf = wencf.tile([KP, nkc, ENC], F32, tag="wf",
                                    name=f"wf{g}",
                                    padded_shape=[KP, WENC_CHUNK, ENC])
                    src = wenc_in[base * KP:(base + nkc) * KP, :]
                    nc.sync.dma_start(wf[:], src.rearrange("(c p) d -> p c d", p=KP))
                    xtf = xtfp.tile([KP, nkc, B], F32, tag="xtf",
                                    name=f"xtf{g}",
                                    padded_shape=[KP, WENC_CHUNK, B])
                    nc.gpsimd.dma_start(
                        xtf[:],
                        xt_in[base * KP:(base + nkc) * KP, :]
                        .rearrange("(c p) b -> p c b", p=KP))
                    if ENC_SPLIT:
                        wr = wencr.tile([KP, nkc, ENC], F32R, tag="wr",
                                        name=f"wr{g}",
                                        padded_shape=[KP, WENC_CHUNK, ENC])
                        wc = wencc.tile([KP, nkc, ENC], F32R, tag="wc",
                                        name=f"wc{g}",
                                        padded_shape=[KP, WENC_CHUNK, ENC])
                        nc.scalar.copy(wr[:], wf[:])
                        nc.vector.tensor_tensor(out=wc[:], in0=wf[:],
                                                in1=wr[:].bitcast(F32),
                                                op=mybir.AluOpType.subtract)
                        xtr = xtrp.tile([KP, nkc, B], F32R, tag="xtr",
                                        name=f"xtr{g}",
                                        padded_shape=[KP, WENC_CHUNK, B])
                        xtc = xtcp.tile([KP, nkc, B], F32R, tag="xtc",
                                        name=f"xtc{g}",
                                        padded_shape=[KP, WENC_CHUNK, B])
                        nc.scalar.copy(xtr[:], xtf[:])
                        nc.vector.tensor_tensor(out=xtc[:], in0=xtf[:],
                                                in1=xtr[:].bitcast(F32),
                                                op=mybir.AluOpType.subtract)
                    for lc in range(nkc):
                        kc = base + lc
                        if ENC_SPLIT:
                            xTr = xtr[:, lc, :]
                            xTc = xtc[:, lc, :]
                        else:
                            xTr = xtf[:, lc, :]
                        for m in range(6):
                            msl = slice(m * 128, (m + 1) * 128)
                            if ENC_SPLIT:
                                nc.tensor.matmul(
                                    feat_ps[m], wr[:, lc, msl], xTr,
                                    start=(kc == 0 and m % 2 == 0), stop=False,
                                    skip_group_check=True)
                                nc.tensor.matmul(
                                    feat_ps[m], wr[:, lc, msl], xTc,
                                    start=False, stop=False,
                                    skip_group_check=True)
                                nc.tensor.matmul(
                                    feat_ps[m], wc[:, lc, msl], xTr,
                                    start=False, stop=(kc == KC - 1),
                                    skip_group_check=True)
                            else:
                                nc.tensor.matmul(
                                    feat_ps[m], wf[:, lc, msl], xTr,
                                    start=(kc == 0 and m % 2 == 0),
                                    stop=(kc == KC - 1),
                                    skip_group_check=True)
                    base += nkc

                featT_sb = []
                for m in range(6):
                    t_ = featp.tile([128, B], F32, tag=f"featT{m}",
                                    name=f"featT_sb{m}")
                    nc.scalar.copy(t_[:], feat_ps[m])
                    featT_sb.append(t_)

                # partial zT = W_lat.T @ featT  (fp32 exact)
                ps_z = ps_small.tile([128, B], F32, tag="small", name="ps_z")
                for m in range(6):
                    nc.tensor.matmul(
                        ps_z[:], wlat_sb[:, m, :], featT_sb[m][:],
                        start=(m == 0), stop=(m == 5),
                    )
                nc.vector.tensor_copy(zTp_sb[:], ps_z[:])

            # prefetch decoder weight staging chunks (scalar HWDGE queue);
            # traced post-encoder so they don't contend during DMA priming.
            # Layout: 9 column-groups of 375 cols (3 col-chunks of 125), each
            # as two enc-half tiles [128, 3, 375].
            wfs = []
            for g2 in range(9):
                csl = slice(g2 * 3 * KP, (g2 + 1) * 3 * KP)
                wf0 = wd2f.tile([128, 3, 3 * KP], F32, tag="wf", name=f"w2f{g2}a")
                wf1 = wd2f.tile([128, 3, 3 * KP], F32, tag="wf2", name=f"w2f{g2}b")
                nc.sync.dma_start(
                    wf0[:], wd2_in[0:384, csl].rearrange("(c p) n -> p c n", p=128))
                nc.sync.dma_start(
                    wf1[:], wd2_in[384:768, csl].rearrange("(c p) n -> p c n", p=128))
                wfs.append((wf0, wf1))

            # ---------------- AllReduce of partial zT ----------------
            bounce_in = dram.tile([128, B], F32)
            bounce_out = dram.tile([128, B], F32)
            nc.scalar.dma_start(bounce_in[:], zTp_sb[:])
            if with_collective:
                nc.gpsimd.collective_compute(
                    "AllReduce",
                    mybir.AluOpType.add,
                    replica_groups=[list(range(NCORES))],
                    ins=[bounce_in.opt()],
                    outs=[bounce_out.opt()],
                )
            else:
                # sim-only stand-in with the same data movement shape
                nc.sync.dma_start(bounce_out[:], bounce_in[:])
            zTr_sb = work.tile([128, B], F32, tag="zTr")
            nc.scalar.dma_start(zTr_sb[:], bounce_out[:])

            # add b_lat (per-partition): column via K=1 matmul transpose
            ps_bl = ps_small.tile([128, 1], F32, tag="small", name="ps_bl")
            nc.tensor.matmul(ps_bl[:], blat_sb[0:1, :], one1[0:1, 0:1],
                             start=True, stop=True)
            blatc_sb = work.tile([128, 1], F32, tag="blatc")
            nc.vector.tensor_copy(blatc_sb[:], ps_bl[:])
            zT_sb = work.tile([128, B], F32, tag="zT")
            nc.vector.tensor_scalar_add(zT_sb[:], zTr_sb[:], blatc_sb[:, 0:1])
            zT2_sb = work.tile([128, B], F32, tag="zT2")
            nc.vector.tensor_scalar_mul(zT2_sb[:], zT_sb[:], 2.0)

            # ---------------- VQ: scores, argmin, gather, losses -------------
            qT_sb = work.tile([128, B], F32, tag="qT")
            with (
                tc.tile_pool(name="ps_d", bufs=2, space="PSUM") as ps_d_pool,
            ):
                for bt in range(2):
                    ps_d = ps_d_pool.tile([128, K], F32, tag="d", name=f"ps_d{bt}")
                    nc.tensor.matmul(ps_d[:], ones_row[0:1, :], nege2_sb[0:1, :],
                                     start=True, stop=False)
                    nc.tensor.matmul(
                        ps_d[:], zT2_sb[:, bt * 128:(bt + 1) * 128], cbT_sb[:],
                        start=False, stop=True,
                    )
                    max8 = rot.tile([128, 8], F32, tag="max8", name=f"max8_{bt}")
                    idx8 = rot.tile([128, 8], mybir.dt.uint32, tag="idx8",
                                    name=f"idx8_{bt}")
                    nc.vector.max(max8[:], ps_d[:])
                    nc.vector.max_index(idx8[:], max8[:], ps_d[:])
                    idx_i32 = rot.tile([128, 1], mybir.dt.int32, tag="idxi",
                                       name=f"idx_i32_{bt}")
                    nc.vector.tensor_copy(idx_i32[:], idx8[:, 0:1])
                    nc.scalar.dma_start(
                        idx_out[bt * 128:(bt + 1) * 128], idx_i32[:, 0]
                    )

                    quant_sb = rot.tile([128, LAT], F32, tag="quant",
                                        name=f"quant{bt}")
                    nc.gpsimd.indirect_dma_start(
                        out=quant_sb[:],
                        out_offset=None,
                        in_=cb_in[:],
                        in_offset=bass.IndirectOffsetOnAxis(ap=idx_i32[:, :1], axis=0),
                    )
                    # z in batch-major for this tile
                    ps_zb = ps_small.tile([128, 128], F32, tag="small",
                                          name=f"ps_zb{bt}")
                    nc.tensor.transpose(
                        ps_zb[:], zT_sb[:, bt * 128:(bt + 1) * 128], ident[:]
                    )
                    diffq = rot.tile([128, LAT], F32, tag="diffq", name=f"diffq{bt}")
                    nc.vector.tensor_tensor(
                        out=diffq[:], in0=quant_sb[:], in1=ps_zb[:],
                        op=mybir.AluOpType.subtract,
                    )
                    sqq = rot.tile([128, LAT], F32, tag="sqq", name=f"sqq{bt}")
                    nc.scalar.activation(
                        out=sqq[:], in_=diffq[:],
                        func=mybir.ActivationFunctionType.Square,
                        accum_out=vacc[:, bt:bt + 1],
                    )
                    # straight-through z_q = z + (quant - z), to match reference fp
                    zq = rot.tile([128, LAT], F32, tag="zq", name=f"zq{bt}")
                    nc.vector.tensor_tensor(
                        out=zq[:], in0=ps_zb[:], in1=diffq[:], op=mybir.AluOpType.add
                    )
                    ps_qt = ps_small.tile([128, 128], F32, tag="small",
                                          name=f"ps_qt{bt}")
                    nc.tensor.transpose(ps_qt[:], zq[:], ident[:])
                    nc.scalar.copy(qT_sb[:, bt * 128:(bt + 1) * 128], ps_qt[:])

            # ---------------- decoder ----------------
            with (
                tc.tile_pool(name="hp", bufs=1) as hp,
                tc.tile_pool(name="wd2r", bufs=2) as wd2r,
                tc.tile_pool(name="wd2c", bufs=2) as wd2c,
            ):
                # layer 1: hT = relu(W_d1.T zqT + b_d1), fp32; split to f32r
                hr_sb, hc_sb = [], []
                with tc.tile_pool(name="ps_h", bufs=1, space="PSUM") as ps_h:
                    h_banks = [
                        ps_h.tile([128, 512], F32, tag=f"hb{i}", name=f"h_bank{i}")
                        for i in range(3)
                    ]
                    h_ps = [
                        h_banks[m // 2][:, (m % 2) * 256:(m % 2) * 256 + 256]
                        for m in range(6)
                    ]
                    for m in range(6):
                        nc.tensor.matmul(
                            h_ps[m], wd1_sb[:, m * 128:(m + 1) * 128], qT_sb[:],
                            start=(m % 2 == 0), stop=(m % 2 == 1),
                            skip_group_check=True,
                        )
                    for m in range(6):
                        hf = hp.tile([128, B], F32, tag=f"hf{m}", name=f"hf{m}")
                        nc.scalar.activation(
                            hf[:], h_ps[m], mybir.ActivationFunctionType.Relu,
                            bias=bd1c[:, m:m + 1],
                        )
                        if DEC_SPLIT:
                            hr = hp.tile([128, B], F32R, tag=f"hr{m}", name=f"hr{m}")
                            hc = hp.tile([128, B], F32R, tag=f"hc{m}", name=f"hc{m}")
                            nc.scalar.copy(hr[:], hf[:])
                            nc.vector.tensor_tensor(out=hc[:], in0=hf[:],
                                                    in1=hr[:].bitcast(F32),
                                                    op=mybir.AluOpType.subtract)
                            hr_sb.append(hr)
                            hc_sb.append(hc)
                        else:
                            hr_sb.append(hf)

                # layer 2 + recon SSE, 3-pass f32r, transposed output
                # xrT[col, b]: weights stationary, h streams at N=256; bias is
                # per-partition; the loss diff reuses the xt input layout.
                with (
                    tc.tile_pool(name="xt2", bufs=2) as xt2p,
                    tc.tile_pool(name="stg", bufs=1) as stgp,
                    tc.tile_pool(name="ps_x", bufs=3, space="PSUM") as ps_x_pool,
                ):
                    stage = stgp.tile([KP, KC, B], F32, tag="stage")
                    for g2 in range(9):
                        wf0, wf1 = wfs[g2]
                        if DEC_SPLIT:
                            w2r = wd2r.tile([128, 6, 3 * KP], F32R, tag="wr",
                                            name=f"w2r{g2}")
                            w2c = wd2c.tile([128, 6, 3 * KP], F32R, tag="wc",
                                            name=f"w2c{g2}")
                            for h, wfh in enumerate((wf0, wf1)):
                                hsl = slice(h * 3, h * 3 + 3)
                                nc.scalar.copy(w2r[:, hsl, :], wfh[:])
                                nc.vector.tensor_tensor(
                                    out=w2c[:, hsl, :], in0=wfh[:],
                                    in1=w2r[:, hsl, :].bitcast(F32),
                                    op=mybir.AluOpType.subtract)
                        xt2 = xt2p.tile([KP, 3, B], F32, tag="xt2",
                                        name=f"xt2_{g2}")
                        nc.gpsimd.dma_start(
                            xt2[:],
                            xt_in[g2 * 3 * KP:(g2 + 1) * 3 * KP, :]
                            .rearrange("(c p) b -> p c b", p=KP))
                        for lc2 in range(3):
                            c2 = g2 * 3 + lc2
                            cs = slice(lc2 * KP, (lc2 + 1) * KP)
                            ps_x = ps_x_pool.tile([KP, B], F32, tag="x",
                                                  name=f"ps_x{c2}")
                            if DEC_SPLIT:
                                for m in range(6):
                                    nc.tensor.matmul(
                                        ps_x[:], w2r[:, m, cs], hr_sb[m][:],
                                        start=(m == 0), stop=False)
                                    nc.tensor.matmul(
                                        ps_x[:], w2c[:, m, cs], hr_sb[m][:],
                                        start=False, stop=False)
                                    nc.tensor.matmul(
                                        ps_x[:], w2r[:, m, cs], hc_sb[m][:],
                                        start=False, stop=(m == 5))
                            else:
                                for m in range(6):
                                    wsrc = (wf0 if m < 3 else wf1)[:, m % 3, cs]
                                    nc.tensor.matmul(
                                        ps_x[:], wsrc, hr_sb[m][:],
                                        start=(m == 0), stop=(m == 5))
                            # add b_d2 (per-partition) while staging for output
                            nc.scalar.activation(
                                stage[:, c2, :], ps_x[:],
                                mybir.ActivationFunctionType.Identity,
                                bias=bd2c_sb[:, c2:c2 + 1],
                            )
                            diff = rot.tile([KP, B], F32, tag="diff",
                                            name=f"diff{c2}")
                            nc.vector.tensor_tensor(
                                out=diff[:], in0=stage[:, c2, :],
                                in1=xt2[:, lc2, :],
                                op=mybir.AluOpType.subtract,
                            )
                            sq = rot.tile([KP, B], F32, tag="sq", name=f"sq{c2}")
                            nc.scalar.activation(
                                out=sq[:], in_=diff[:],
                                func=mybir.ActivationFunctionType.Square,
                                accum_out=racc[0:KP, c2:c2 + 1],
                            )
                        # write back this column-group (9 chunks -> 3 DMAs)
                        if g2 % 3 == 2:
                            gs = slice((g2 - 2) * 3, (g2 + 1) * 3)
                            nc.sync.dma_start(
                                xrt_out[(g2 - 2) * 3 * KP:(g2 + 1) * 3 * KP, :]
                                .rearrange("(c p) b -> p c b", p=KP),
                                stage[:, gs, :],
                            )

            # ---------------- final scalar reductions ----------------
            rsum = work.tile([128, 1], F32, tag="rsum")
            nc.vector.tensor_reduce(
                out=rsum[0:KP, :1], in_=racc[0:KP, :KC], axis=mybir.AxisListType.X,
                op=mybir.AluOpType.add,
            )
            vsum = work.tile([128, 1], F32, tag="vsum")
            nc.vector.tensor_reduce(
                out=vsum[:, :1], in_=vacc[:, :2], axis=mybir.AxisListType.X,
                op=mybir.AluOpType.add,
            )
            ps_r = ps_small.tile([1, 1], F32, tag="small", name="ps_r")
            nc.tensor.matmul(ps_r[:], rsum[0:KP, 0:1], ones_col[0:KP, 0:1],
                             start=True, stop=True)
            r_sb = work.tile([1, 1], F32, tag="r_sb")
            nc.scalar.copy(r_sb[:], ps_r[:])
            nc.sync.dma_start(rsse_out[:], r_sb[:])
            ps_v = ps_small.tile([1, 1], F32, tag="small", name="ps_v")
            nc.tensor.matmul(ps_v[:], vsum[:, 0:1], ones_col[:, 0:1],
                             start=True, stop=True)
            v_sb = work.tile([1, 1], F32, tag="v_sb")
            nc.scalar.copy(v_sb[:], ps_v[:])
            nc.sync.dma_start(vsse_out[:], v_sb[:])

    nc.compile()
    return nc


_NC_CACHE = None


def _get_nc():
    global _NC_CACHE
    if _NC_CACHE is None:
        _NC_CACHE = build_nc()
    return _NC_CACHE


def run_sharded(inputs, trace=False, trace_kwargs=None):
    """Run the SPMD kernel; returns (results_list, BassKernelResults)."""
    x = np.ascontiguousarray(np.asarray(inputs["x"], dtype=np.float32)).reshape(B, IN_DIM)
    W_enc = np.asarray(inputs["W_enc"], dtype=np.float32)
    W_lat = np.asarray(inputs["W_lat"], dtype=np.float32)
    b_lat = np.asarray(inputs["b_lat"], dtype=np.float32).reshape(1, LAT)
    codebook = np.asarray(inputs["codebook"], dtype=np.float32)
    W_d1 = np.asarray(inputs["W_d1"], dtype=np.float32)
    b_d1 = np.asarray(inputs["b_d1"], dtype=np.float32).reshape(1, ENC)
    W_d2 = np.asarray(inputs["W_d2"], dtype=np.float32)
    b_d2 = np.asarray(inputs["b_d2"], dtype=np.float32).reshape(1, IN_DIM)

    nc = _get_nc()
    in_maps = []
    for c in range(NCORES):
        sl = slice(c * SLICE, (c + 1) * SLICE)
        in_maps.append({
            "x_slice": np.ascontiguousarray(x[:, sl]),
            "xt_slice": np.ascontiguousarray(x[:, sl].T),
            "wenc_slice": np.ascontiguousarray(W_enc[sl, :]),
            "wlat": W_lat,
            "blat": b_lat,
            "codebook": codebook,
            "wd1": W_d1,
            "bd1": b_d1,
            "wd2_slice": np.ascontiguousarray(W_d2[:, sl]),
            "bd2_slice": np.ascontiguousarray(b_d2[:, sl]),
        })
    import time as _time

    last_exc = None
    for attempt in range(3):
        try:
            res = run_bass_kernel_spmd(
                nc, in_maps, core_ids=list(range(NCORES)), trace=trace,
                trace_kwargs=trace_kwargs or {},
            )
            return res.results, res
        except Exception as e:  # transient axon/NRT faults recover on retry
            last_exc = e
            _time.sleep(10 * (attempt + 1))
    raise last_exc


def kernel(**inputs):
    results, _ = run_sharded(inputs)

    xrt = np.concatenate([results[c]["xrt"] for c in range(NCORES)], axis=0)
    x_recon = np.ascontiguousarray(xrt.T).reshape(B, LEADS, T)

    recon_sse = float(np.sum([np.float64(results[c]["recon_sse"][0, 0])
                              for c in range(NCORES)]))
    recon_loss = np.float32(recon_sse / (B * IN_DIM))

    m = np.float32(np.float64(results[0]["vq_sse"][0, 0]) / (B * LAT))
    # reference: loss = q_latent + COMMIT * e_latent, both numerically == m
    vq_loss = np.float32(m + np.float32(COMMIT) * m)
    total = np.float32(recon_loss + vq_loss)

    indices = results[0]["indices"].astype(np.int32)
    return x_recon, total, vq_loss, indices


# revision 25
# speedup vs baseline: 1.0034x; 1.0034x over previous
"""Trainium2 Bass kernel for EncoderVQVAEFixed (VQ-VAE forward pass).

Shards the 27000-dim encoder contraction / decoder output dim across 8
NeuronCores (tensor-parallel), all-reduces the tiny latent z, replicates the
VQ + first decoder layer, and re-shards the big decoder matmul column-wise.

The two large matmuls (x @ W_enc and h @ W_d2) run as 3-pass float32r
split-precision products (a=ar+ac, b=br+bc; out = ar@br + ar@bc + ac@br),
which matches fp32 accuracy (~2e-7) at 3 PE cycles/row instead of fp32's 4.
"""

import os
import sys

sys.path.insert(0, "/opt/trn_rl_repo")

import numpy as np

import concourse.bass as bass
import concourse.bacc as bacc
import concourse.tile as tile
from concourse import mybir
from concourse.bass_utils import run_bass_kernel_spmd
from concourse.masks import make_identity

NCORES = 8
B = 256
LEADS, T = 12, 2250
IN_DIM = LEADS * T          # 27000
ENC = 768
LAT = 128
K = 512
COMMIT = 0.1
SLICE = IN_DIM // NCORES    # 3375
KC = 27                     # contraction tiles per core
KP = 125                    # partition size of each contraction tile (27*125=3375)
WENC_CHUNK = 3              # kc per wenc DMA chunk (9 chunks)
# last chunk is 303 wide; f32r matmuls need aligned free dims, so pad to 304
# with a zeroed column (nn = padded width, nv = valid width)
N_CHUNKS = [(0, 512, 512), (512, 512, 512), (1024, 512, 512), (1536, 512, 512),
            (2048, 512, 512), (2560, 512, 512), (3072, 304, 303)]
F32 = mybir.dt.float32
F32R = mybir.dt.float32r
ENC_SPLIT = os.environ.get("ENC_SPLIT", "1") == "1"
DEC_SPLIT = os.environ.get("DEC_SPLIT", "1") == "1"


def build_nc(with_collective=True):
    nc = bacc.Bacc(None, target_bir_lowering=False, debug=False, num_devices=NCORES)

    x_in = nc.dram_tensor("x_slice", [B, SLICE], F32, kind="ExternalInput")
    xt_in = nc.dram_tensor("xt_slice", [SLICE, B], F32, kind="ExternalInput")
    wenc_in = nc.dram_tensor("wenc_slice", [SLICE, ENC], F32, kind="ExternalInput")
    wlat_in = nc.dram_tensor("wlat", [ENC, LAT], F32, kind="ExternalInput")
    blat_in = nc.dram_tensor("blat", [1, LAT], F32, kind="ExternalInput")
    cb_in = nc.dram_tensor("codebook", [K, LAT], F32, kind="ExternalInput")
    wd1_in = nc.dram_tensor("wd1", [LAT, ENC], F32, kind="ExternalInput")
    bd1_in = nc.dram_tensor("bd1", [1, ENC], F32, kind="ExternalInput")
    wd2_in = nc.dram_tensor("wd2_slice", [ENC, SLICE], F32, kind="ExternalInput")
    bd2_in = nc.dram_tensor("bd2_slice", [1, SLICE], F32, kind="ExternalInput")

    xrt_out = nc.dram_tensor("xrt", [SLICE, B], F32, kind="ExternalOutput")
    idx_out = nc.dram_tensor("indices", [B], mybir.dt.int32, kind="ExternalOutput")
    rsse_out = nc.dram_tensor("recon_sse", [1, 1], F32, kind="ExternalOutput")
    vsse_out = nc.dram_tensor("vq_sse", [1, 1], F32, kind="ExternalOutput")

    with tile.TileContext(nc) as tc:
        with (
            tc.tile_pool(name="const", bufs=1) as cpool,
            tc.tile_pool(name="xp", bufs=1) as xpool,
            tc.tile_pool(name="work", bufs=1) as work,
            tc.tile_pool(name="rot", bufs=2) as rot,
            tc.tile_pool(name="wd2f", bufs=4) as wd2f,
            tc.tile_pool(name="vqp", bufs=1) as vqp,
            tc.tile_pool(name="ps_small", bufs=4, space="PSUM") as ps_small,
            tc.tile_pool(name="dram", bufs=1, space="DRAM") as dram,
        ):
            # ---------------- constants / small inputs ----------------
            ident = cpool.tile([128, 128], F32, tag="ident")
            make_identity(nc, ident[:])
            ones_col = cpool.tile([128, 1], F32, tag="ones_col")
            nc.gpsimd.memset(ones_col[:], 1.0)
            ones_row = cpool.tile([1, 128], F32, tag="ones_row")
            nc.gpsimd.memset(ones_row[:], 1.0)
            one1 = cpool.tile([1, 1], F32, tag="one1")
            nc.gpsimd.memset(one1[:], 1.0)

            wlat_sb = cpool.tile([128, 6, LAT], F32, tag="wlat")
            nc.sync.dma_start(
                wlat_sb[:], wlat_in[:].rearrange("(c p) l -> p c l", p=128)
            )
            blat_sb = cpool.tile([1, LAT], F32, tag="blat")
            nc.sync.dma_start(blat_sb[:], blat_in[:])
            cb_sb = cpool.tile([128, 4, LAT], F32, tag="cb")
            nc.sync.dma_start(cb_sb[:], cb_in[:].rearrange("(c p) l -> p c l", p=128))
            wd1_sb = cpool.tile([LAT, ENC], F32, tag="wd1")
            nc.sync.dma_start(wd1_sb[:], wd1_in[:])
            bd1_sb = cpool.tile([1, ENC], F32, tag="bd1")
            nc.sync.dma_start(bd1_sb[:], bd1_in[:])
            # b_d2 as per-partition columns for the transposed decoder output
            bd2c_sb = cpool.tile([KP, KC], F32, tag="bd2c")
            nc.gpsimd.dma_start(
                bd2c_sb[:], bd2_in[:].rearrange("o (c p) -> (o p) c", p=KP))


            # b_d1 as per-partition columns (independent of everything else)
            bd1c = work.tile([128, 6], F32, tag="bd1c")
            for m in range(6):
                ps_b = ps_small.tile([128, 1], F32, tag="small", name=f"ps_bd1_{m}")
                nc.tensor.matmul(ps_b[:], bd1_sb[0:1, m * 128:(m + 1) * 128],
                                 one1[0:1, 0:1], start=True, stop=True)
                nc.vector.tensor_copy(bd1c[:, m:m + 1], ps_b[:])

            # loss accumulators
            racc = work.tile([128, KC], F32, tag="racc")
            vacc = work.tile([128, 2], F32, tag="vacc")

            # codebook transpose + e2 row: independent of z, runs during encoder
            cbT_sb = vqp.tile([128, K], F32, tag="cbT")
            for c in range(4):
                ps_t = ps_small.tile([128, 128], F32, tag="small",
                                     name=f"ps_cbt{c}")
                nc.tensor.transpose(ps_t[:], cb_sb[:, c, :], ident[:])
                nc.scalar.copy(cbT_sb[:, c * 128:(c + 1) * 128], ps_t[:])
            cbsqT_sb = vqp.tile([128, K], F32, tag="cbsqT")
            nc.vector.tensor_tensor(out=cbsqT_sb[:], in0=cbT_sb[:],
                                    in1=cbT_sb[:], op=mybir.AluOpType.mult)
            ps_e2 = ps_small.tile([1, K], F32, tag="small", name="ps_e2")
            nc.tensor.matmul(ps_e2[:], ones_col[:, 0:1], cbsqT_sb[:],
                             start=True, stop=True)
            nege2_sb = vqp.tile([1, K], F32, tag="nege2")
            nc.scalar.mul(nege2_sb[:], ps_e2[:], -1.0)

            # ---------------- encoder: featT[768, 256], 3-pass f32r ----------
            zTp_sb = work.tile([128, B], F32, tag="zTp")
            with (
                tc.tile_pool(name="wencf", bufs=3) as wencf,
                tc.tile_pool(name="wencr", bufs=3) as wencr,
                tc.tile_pool(name="wencc", bufs=3) as wencc,
                tc.tile_pool(name="xtf", bufs=2) as xtfp,
                tc.tile_pool(name="xtr", bufs=2) as xtrp,
                tc.tile_pool(name="xtc", bufs=2) as xtcp,
                tc.tile_pool(name="featp", bufs=1) as featp,
                tc.tile_pool(name="ps_feat", bufs=1, space="PSUM") as ps_feat,
            ):
                # 6 logical [128, 256] accumulators packed 2-per-bank
                feat_banks = [
                    ps_feat.tile([128, 512], F32, tag=f"fb{i}", name=f"feat_bank{i}")
                    for i in range(3)
                ]
                feat_ps = [
                    feat_banks[m // 2][:, (m % 2) * 256:(m % 2) * 256 + 256]
                    for m in range(6)
                ]
                # small leading groups so the first matmuls start early
                groups = [1, 2] + [3] * 8
                base = 0
                for g, nkc in enumerate(groups):
                    wf = wencf.tile([KP, nkc, ENC], F32, tag="wf",
                                    name=f"wf{g}",
                                    padded_shape=[KP, WENC_CHUNK, ENC])
                    src = wenc_in[base * KP:(base + nkc) * KP, :]
                    nc.sync.dma_start(wf[:], src.rearrange("(c p) d -> p c d", p=KP))
                    xtf = xtfp.tile([KP, nkc, B], F32, tag="xtf",
                                    name=f"xtf{g}",
                                    padded_shape=[KP, WENC_CHUNK, B])
                    nc.gpsimd.dma_start(
                        xtf[:],
                        xt_in[base * KP:(base + nkc) * KP, :]
                        .rearrange("(c p) b -> p c b", p=KP))
                    if ENC_SPLIT:
                        wr = wencr.tile([KP, nkc, ENC], F32R, tag="wr",
                                        name=f"wr{g}",
                                        padded_shape=[KP, WENC_CHUNK, ENC])
                        wc = wencc.tile([KP, nkc, ENC], F32R, tag="wc",
                                        name=f"wc{g}",
                                        padded_shape=[KP, WENC_CHUNK, ENC])
                        nc.scalar.copy(wr[:], wf[:])
                        nc.vector.tensor_tensor(out=wc[:], in0=wf[:],
                                                in1=wr[:].bitcast(F32),
                                                op=mybir.AluOpType.subtract)
                        xtr = xtrp.tile([KP, nkc, B], F32R, tag="xtr",
                                        name=f"xtr{g}",
                                        padded_shape=[KP, WENC_CHUNK, B])
                        xtc = xtcp.tile([KP, nkc, B], F32R, tag="xtc",
                                        name=f"xtc{g}",
                                        padded_shape=[KP, WENC_CHUNK, B])
                        nc.scalar.copy(xtr[:], xtf[:])
                        nc.vector.tensor_tensor(out=xtc[:], in0=xtf[:],
                                                in1=xtr[:].bitcast(F32),
                                                op=mybir.AluOpType.subtract)
                    for lc in range(nkc):
                        kc = base + lc
                        if ENC_SPLIT:
                            xTr = xtr[:, lc, :]
                            xTc = xtc[:, lc, :]
                        else:
                            xTr = xtf[:, lc, :]
                        for m in range(6):
                            msl = slice(m * 128, (m + 1) * 128)
                            if ENC_SPLIT:
                                nc.tensor.matmul(
                                    feat_ps[m], wr[:, lc, msl], xTr,
                                    start=(kc == 0 and m % 2 == 0), stop=False,
                                    skip_group_check=True)
                                nc.tensor.matmul(
                                    feat_ps[m], wr[:, lc, msl], xTc,
                                    start=False, stop=False,
                                    skip_group_check=True)
                                nc.tensor.matmul(
                                    feat_ps[m], wc[:, lc, msl], xTr,
                                    start=False, stop=(kc == KC - 1),
                                    skip_group_check=True)
                            else:
                                nc.tensor.matmul(
                                    feat_ps[m], wf[:, lc, msl], xTr,
                                    start=(kc == 0 and m % 2 == 0),
                                    stop=(kc == KC - 1),
                                    skip_group_check=True)
                    base += nkc

                featT_sb = []
                for m in range(6):
                    t_ = featp.tile([128, B], F32, tag=f"featT{m}",
                                    name=f"featT_sb{m}")
                    nc.scalar.copy(t_[:], feat_ps[m])
                    featT_sb.append(t_)

                # partial zT = W_lat.T @ featT  (fp32 exact)
                ps_z = ps_small.tile([128, B], F32, tag="small", name="ps_z")
                for m in range(6):
                    nc.tensor.matmul(
                        ps_z[:], wlat_sb[:, m, :], featT_sb[m][:],
                        start=(m == 0), stop=(m == 5),
                    )
                nc.vector.tensor_copy(zTp_sb[:], ps_z[:])

            # prefetch decoder weight staging chunks (scalar HWDGE queue);
            # traced post-encoder so they don't contend during DMA priming.
            # Layout: 9 column-groups of 375 cols (3 col-chunks of 125), each
            # as two enc-half tiles [128, 3, 375].
            wfs = []
            for g2 in range(9):
                csl = slice(g2 * 3 * KP, (g2 + 1) * 3 * KP)
                wf0 = wd2f.tile([128, 3, 3 * KP], F32, tag="wf", name=f"w2f{g2}a")
                wf1 = wd2f.tile([128, 3, 3 * KP], F32, tag="wf2", name=f"w2f{g2}b")
                nc.sync.dma_start(
                    wf0[:], wd2_in[0:384, csl].rearrange("(c p) n -> p c n", p=128))
                nc.sync.dma_start(
                    wf1[:], wd2_in[384:768, csl].rearrange("(c p) n -> p c n", p=128))
                wfs.append((wf0, wf1))

            # ---------------- AllReduce of partial zT ----------------
            bounce_in = dram.tile([128, B], F32)
            bounce_out = dram.tile([128, B], F32)
            nc.scalar.dma_start(bounce_in[:], zTp_sb[:])
            if with_collective:
                nc.gpsimd.collective_compute(
                    "AllReduce",
                    mybir.AluOpType.add,
                    replica_groups=[list(range(NCORES))],
                    ins=[bounce_in.opt()],
                    outs=[bounce_out.opt()],
                )
            else:
                # sim-only stand-in with the same data movement shape
                nc.sync.dma_start(bounce_out[:], bounce_in[:])
            zTr_sb = work.tile([128, B], F32, tag="zTr")
            nc.scalar.dma_start(zTr_sb[:], bounce_out[:])

            # add b_lat (per-partition): column via K=1 matmul transpose
            ps_bl = ps_small.tile([128, 1], F32, tag="small", name="ps_bl")
            nc.tensor.matmul(ps_bl[:], blat_sb[0:1, :], one1[0:1, 0:1],
                             start=True, stop=True)
            blatc_sb = work.tile([128, 1], F32, tag="blatc")
            nc.vector.tensor_copy(blatc_sb[:], ps_bl[:])
            zT_sb = work.tile([128, B], F32, tag="zT")
            nc.vector.tensor_scalar_add(zT_sb[:], zTr_sb[:], blatc_sb[:, 0:1])
            zT2_sb = work.tile([128, B], F32, tag="zT2")
            nc.vector.tensor_scalar_mul(zT2_sb[:], zT_sb[:], 2.0)

            # ---------------- VQ: scores, argmin, gather, losses -------------
            qT_sb = work.tile([128, B], F32, tag="qT")
            with (
                tc.tile_pool(name="ps_d", bufs=2, space="PSUM") as ps_d_pool,
            ):
                for bt in range(2):
                    ps_d = ps_d_pool.tile([128, K], F32, tag="d", name=f"ps_d{bt}")
                    nc.tensor.matmul(ps_d[:], ones_row[0:1, :], nege2_sb[0:1, :],
                                     start=True, stop=False)
                    nc.tensor.matmul(
                        ps_d[:], zT2_sb[:, bt * 128:(bt + 1) * 128], cbT_sb[:],
                        start=False, stop=True,
                    )
                    max8 = rot.tile([128, 8], F32, tag="max8", name=f"max8_{bt}")
                    idx8 = rot.tile([128, 8], mybir.dt.uint32, tag="idx8",
                                    name=f"idx8_{bt}")
                    nc.vector.max(max8[:], ps_d[:])
                    nc.vector.max_index(idx8[:], max8[:], ps_d[:])
                    idx_i32 = rot.tile([128, 1], mybir.dt.int32, tag="idxi",
                                       name=f"idx_i32_{bt}")
                    nc.vector.tensor_copy(idx_i32[:], idx8[:, 0:1])
                    nc.scalar.dma_start(
                        idx_out[bt * 128:(bt + 1) * 128], idx_i32[:, 0]
                    )

                    quant_sb = rot.tile([128, LAT], F32, tag="quant",
                                        name=f"quant{bt}")
                    nc.gpsimd.indirect_dma_start(
                        out=quant_sb[:],
                        out_offset=None,
                        in_=cb_in[:],
                        in_offset=bass.IndirectOffsetOnAxis(ap=idx_i32[:, :1], axis=0),
                    )
                    # z in batch-major for this tile
                    ps_zb = ps_small.tile([128, 128], F32, tag="small",
                                          name=f"ps_zb{bt}")
                    nc.tensor.transpose(
                        ps_zb[:], zT_sb[:, bt * 128:(bt + 1) * 128], ident[:]
                    )
                    diffq = rot.tile([128, LAT], F32, tag="diffq", name=f"diffq{bt}")
                    nc.vector.tensor_tensor(
                        out=diffq[:], in0=quant_sb[:], in1=ps_zb[:],
                        op=mybir.AluOpType.subtract,
                    )
                    sqq = rot.tile([128, LAT], F32, tag="sqq", name=f"sqq{bt}")
                    nc.scalar.activation(
                        out=sqq[:], in_=diffq[:],
                        func=mybir.ActivationFunctionType.Square,
                        accum_out=vacc[:, bt:bt + 1],
                    )
                    # straight-through z_q = z + (quant - z), to match reference fp
                    zq = rot.tile([128, LAT], F32, tag="zq", name=f"zq{bt}")
                    nc.vector.tensor_tensor(
                        out=zq[:], in0=ps_zb[:], in1=diffq[:], op=mybir.AluOpType.add
                    )
                    ps_qt = ps_small.tile([128, 128], F32, tag="small",
                                          name=f"ps_qt{bt}")
                    nc.tensor.transpose(ps_qt[:], zq[:], ident[:])
                    nc.scalar.copy(qT_sb[:, bt * 128:(bt + 1) * 128], ps_qt[:])

            # ---------------- decoder ----------------
            with (
                tc.tile_pool(name="hp", bufs=1) as hp,
                tc.tile_pool(name="wd2r", bufs=2) as wd2r,
                tc.tile_pool(name="wd2c", bufs=2) as wd2c,
            ):
                # layer 1: hT = relu(W_d1.T zqT + b_d1), fp32; split to f32r
                hr_sb, hc_sb = [], []
                with tc.tile_pool(name="ps_h", bufs=1, space="PSUM") as ps_h:
                    h_banks = [
                        ps_h.tile([128, 512], F32, tag=f"hb{i}", name=f"h_bank{i}")
                        for i in range(3)
                    ]
                    h_ps = [
                        h_banks[m // 2][:, (m % 2) * 256:(m % 2) * 256 + 256]
                        for m in range(6)
                    ]
                    for m in range(6):
                        nc.tensor.matmul(
                            h_ps[m], wd1_sb[:, m * 128:(m + 1) * 128], qT_sb[:],
                            start=(m % 2 == 0), stop=(m % 2 == 1),
                            skip_group_check=True,
                        )
                    for m in range(6):
                        hf = hp.tile([128, B], F32, tag=f"hf{m}", name=f"hf{m}")
                        nc.scalar.activation(
                            hf[:], h_ps[m], mybir.ActivationFunctionType.Relu,
                            bias=bd1c[:, m:m + 1],
                        )
                        if DEC_SPLIT:
                            hr = hp.tile([128, B], F32R, tag=f"hr{m}", name=f"hr{m}")
                            hc = hp.tile([128, B], F32R, tag=f"hc{m}", name=f"hc{m}")
                            nc.scalar.copy(hr[:], hf[:])
                            nc.vector.tensor_tensor(out=hc[:], in0=hf[:],
                                                    in1=hr[:].bitcast(F32),
                                                    op=mybir.AluOpType.subtract)
                            hr_sb.append(hr)
                            hc_sb.append(hc)
                        else:
                            hr_sb.append(hf)

                # layer 2 + recon SSE, 3-pass f32r, transposed output
                # xrT[col, b]: weights stationary, h streams at N=256; bias is
                # per-partition; the loss diff reuses the xt input layout.
                with (
                    tc.tile_pool(name="xt2", bufs=2) as xt2p,
                    tc.tile_pool(name="stg", bufs=1) as stgp,
                    tc.tile_pool(name="ps_x", bufs=3, space="PSUM") as ps_x_pool,
                ):
                    stage = stgp.tile([KP, KC, B], F32, tag="stage")
                    for g2 in range(9):
                        wf0, wf1 = wfs[g2]
                        if DEC_SPLIT:
                            w2r = wd2r.tile([128, 6, 3 * KP], F32R, tag="wr",
                                            name=f"w2r{g2}")
                            w2c = wd2c.tile([128, 6, 3 * KP], F32R, tag="wc",
                                            name=f"w2c{g2}")
                            for h, wfh in enumerate((wf0, wf1)):
                                hsl = slice(h * 3, h * 3 + 3)
                                nc.scalar.copy(w2r[:, hsl, :], wfh[:])
                                nc.vector.tensor_tensor(
                                    out=w2c[:, hsl, :], in0=wfh[:],
                                    in1=w2r[:, hsl, :].bitcast(F32),
                                    op=mybir.AluOpType.subtract)
                        xt2 = xt2p.tile([KP, 3, B], F32, tag="xt2",
                                        name=f"xt2_{g2}")
                        nc.gpsimd.dma_start(
                            xt2[:],
                            xt_in[g2 * 3 * KP:(g2 + 1) * 3 * KP, :]
                            .rearrange("(c p) b -> p c b", p=KP))
                        for lc2 in range(3):
                            c2 = g2 * 3 + lc2
                            cs = slice(lc2 * KP, (lc2 + 1) * KP)
                            ps_x = ps_x_pool.tile([KP, B], F32, tag="x",
                                                  name=f"ps_x{c2}")
                            if DEC_SPLIT:
                                for m in range(6):
                                    nc.tensor.matmul(
                                        ps_x[:], w2r[:, m, cs], hr_sb[m][:],
                                        start=(m == 0), stop=False)
                                    nc.tensor.matmul(
                                        ps_x[:], w2c[:, m, cs], hr_sb[m][:],
                                        start=False, stop=False)
                                    nc.tensor.matmul(
                                        ps_x[:], w2r[:, m, cs], hc_sb[m][:],
                                        start=False, stop=(m == 5))
                            else:
                                for m in range(6):
                                    wsrc = (wf0 if m < 3 else wf1)[:, m % 3, cs]
                                    nc.tensor.matmul(
                                        ps_x[:], wsrc, hr_sb[m][:],
                                        start=(m == 0), stop=(m == 5))
                            # add b_d2 (per-partition) while staging for output
                            nc.scalar.activation(
                                stage[:, c2, :], ps_x[:],
                                mybir.ActivationFunctionType.Identity,
                                bias=bd2c_sb[:, c2:c2 + 1],
                            )
                            diff = rot.tile([KP, B], F32, tag="diff",
                                            name=f"diff{c2}")
                            nc.vector.tensor_tensor(
                                out=diff[:], in0=stage[:, c2, :],
                                in1=xt2[:, lc2, :],
                                op=mybir.AluOpType.subtract,
                            )
                            sq = rot.tile([KP, B], F32, tag="sq", name=f"sq{c2}")
                            nc.scalar.activation(
                                out=sq[:], in_=diff[:],
                                func=mybir.ActivationFunctionType.Square,
                                accum_out=racc[0:KP, c2:c2 + 1],
                            )
                        # write back this column-group (9 chunks -> 3 DMAs)
                        if g2 % 3 == 2:
                            gs = slice((g2 - 2) * 3, (g2 + 1) * 3)
                            nc.sync.dma_start(
                                xrt_out[(g2 - 2) * 3 * KP:(g2 + 1) * 3 * KP, :]
                                .rearrange("(c p) b -> p c b", p=KP),
                                stage[:, gs, :],
                            )

            # ---------------- final scalar reductions ----------------
            rsum = work.tile([128, 1], F32, tag="rsum")
            nc.vector.tensor_reduce(
                out=rsum[0:KP, :1], in_=racc[0:KP, :KC], axis=mybir.AxisListType.X,
                op=mybir.AluOpType.add,
            )
            vsum = work.tile([128, 1], F32, tag="vsum")
            nc.vector.tensor_reduce(
                out=vsum[:, :1], in_=vacc[:, :2], axis=mybir.AxisListType.X,
                op=mybir.AluOpType.add,
            )
            ps_r = ps_small.tile([1, 1], F32, tag="small", name="ps_r")
            nc.tensor.matmul(ps_r[:], rsum[0:KP, 0:1], ones_col[0:KP, 0:1],
                             start=True, stop=True)
            r_sb = work.tile([1, 1], F32, tag="r_sb")
            nc.scalar.copy(r_sb[:], ps_r[:])
            nc.sync.dma_start(rsse_out[:], r_sb[:])
            ps_v = ps_small.tile([1, 1], F32, tag="small", name="ps_v")
            nc.tensor.matmul(ps_v[:], vsum[:, 0:1], ones_col[:, 0:1],
                             start=True, stop=True)
            v_sb = work.tile([1, 1], F32, tag="v_sb")
            nc.scalar.copy(v_sb[:], ps_v[:])
            nc.sync.dma_start(vsse_out[:], v_sb[:])

    nc.compile()
    return nc


_NC_CACHE = None


def _get_nc():
    global _NC_CACHE
    if _NC_CACHE is None:
        _NC_CACHE = build_nc()
    return _NC_CACHE


def run_sharded(inputs, trace=False, trace_kwargs=None):
    """Run the SPMD kernel; returns (results_list, BassKernelResults)."""
    x = np.ascontiguousarray(np.asarray(inputs["x"], dtype=np.float32)).reshape(B, IN_DIM)
    W_enc = np.asarray(inputs["W_enc"], dtype=np.float32)
    W_lat = np.asarray(inputs["W_lat"], dtype=np.float32)
    b_lat = np.asarray(inputs["b_lat"], dtype=np.float32).reshape(1, LAT)
    codebook = np.asarray(inputs["codebook"], dtype=np.float32)
    W_d1 = np.asarray(inputs["W_d1"], dtype=np.float32)
    b_d1 = np.asarray(inputs["b_d1"], dtype=np.float32).reshape(1, ENC)
    W_d2 = np.asarray(inputs["W_d2"], dtype=np.float32)
    b_d2 = np.asarray(inputs["b_d2"], dtype=np.float32).reshape(1, IN_DIM)

    nc = _get_nc()
    in_maps = []
    for c in range(NCORES):
        sl = slice(c * SLICE, (c + 1) * SLICE)
        in_maps.append({
            "x_slice": np.ascontiguousarray(x[:, sl]),
            "xt_slice": np.ascontiguousarray(x[:, sl].T),
            "wenc_slice": np.ascontiguousarray(W_enc[sl, :]),
            "wlat": W_lat,
            "blat": b_lat,
            "codebook": codebook,
            "wd1": W_d1,
            "bd1": b_d1,
            "wd2_slice": np.ascontiguousarray(W_d2[:, sl]),
            "bd2_slice": np.ascontiguousarray(b_d2[:, sl]),
        })
    import time as _time

    last_exc = None
    for attempt in range(3):
        try:
            res = run_bass_kernel_spmd(
                nc, in_maps, core_ids=list(range(NCORES)), trace=trace,
                trace_kwargs=trace_kwargs or {},
            )
            return res.results, res
        except Exception as e:  # transient axon/NRT faults recover on retry
            last_exc = e
            _time.sleep(10 * (attempt + 1))
    raise last_exc


def kernel(**inputs):
    results, _ = run_sharded(inputs)

    xrt = np.concatenate([results[c]["xrt"] for c in range(NCORES)], axis=0)
    x_recon = np.ascontiguousarray(xrt.T).reshape(B, LEADS, T)

    recon_sse = float(np.sum([np.float64(results[c]["recon_sse"][0, 0])
                              for c in range(NCORES)]))
    recon_loss = np.float32(recon_sse / (B * IN_DIM))

    m = np.float32(np.float64(results[0]["vq_sse"][0, 0]) / (B * LAT))
    # reference: loss = q_latent + COMMIT * e_latent, both numerically == m
    vq_loss = np.float32(m + np.float32(COMMIT) * m)
    total = np.float32(recon_loss + vq_loss)

    indices = results[0]["indices"].astype(np.int32)
    return x_recon, total, vq_loss, indices
